# revision 1
# baseline (speedup 1.0000x reference)
"""Distributed embedding gather (OEPEmbedding) on 8 Trainium2 NeuronCores.

Strategy (expert-parallel-style dispatch/combine, with host-side routing):
  - Sort all G*T token ids; cut the sorted list into 8 equal chunks
    (perfect load balance). Each core receives a contiguous slice of the
    embedding table that covers its chunk's id span (R rows, indexable by
    int16 local ids as the HW dma_gather instruction requires), plus the
    local ids in dma_gather's wrapped [128, C/16] layout.
  - On device, each core runs chunked dma_gather (HBM table -> SBUF)
    pipelined with HWDGE writes (SBUF -> HBM output).
  - Host inverts the sort permutation to assemble [T, G, D] output.

Degenerate id distributions that don't fit 8 span-limited chunks fall back
to additional SPMD launches (never happens for uniform ids).
"""

import numpy as np

VOCAB = 128000
D = 2048
G = 3
T = 16384
N = G * T            # 49152 total gathers
N_CORES = 8
C = N // N_CORES     # 6144 rows gathered per core (SPMD capacity)
R = 18048            # table rows staged per core (max id span per chunk)
CHUNK = 512          # ids per dma_gather instruction
P = 128
CPP = CHUNK // P     # rows per partition per chunk = 4
N_CHUNKS = C // CHUNK

_cached = None


def _build():
    """Build + compile the SPMD Bass program once per process."""
    global _cached
    if _cached is not None:
        return _cached
    import concourse.bacc as bacc
    import concourse.mybir as mybir
    import concourse.tile as tile

    nc = bacc.Bacc("TRN2", target_bir_lowering=False, debug=False)
    with tile.TileContext(nc) as tc:
        with tc.tile_pool(name="dram", bufs=1, space="DRAM") as dram:
            table = dram.tile([R, D], mybir.dt.float32, kind="ExternalInput")
            idxs = dram.tile([P, C // 16], mybir.dt.int16, kind="ExternalInput")
            out = dram.tile([P, N_CHUNKS * CPP, D], mybir.dt.float32,
                            kind="ExternalOutput")
            with (
                tc.tile_pool(name="idxp", bufs=1) as idxp,
                tc.tile_pool(name="gbuf", bufs=4) as gbuf,
            ):
                idx_sb = idxp.tile([P, C // 16], mybir.dt.int16)
                nc.sync.dma_start(idx_sb[:], idxs[:])
                cols = CHUNK // 16  # idx columns per chunk
                for k in range(N_CHUNKS):
                    t = gbuf.tile([P, CPP, D], mybir.dt.float32)
                    nc.gpsimd.dma_gather(
                        t[:],
                        table[:],
                        idx_sb[:, k * cols:(k + 1) * cols],
                        CHUNK,
                        CHUNK,
                        D,
                    )
                    nc.sync.dma_start(out[:, k * CPP:(k + 1) * CPP, :], t[:])
    nc.compile()
    _cached = (nc, table.name, idxs.name, out.name)
    return _cached


def _wrap_idxs(local_ids):
    """dma_gather idx layout: idx i at [i%16, i//16], replicated to 128 parts."""
    padded = np.zeros(C, np.int16)
    padded[: len(local_ids)] = local_ids
    w16 = padded.reshape(C // 16, 16).T  # [16, C/16]
    return np.ascontiguousarray(np.tile(w16, (8, 1)))  # [128, C/16]


def kernel(input_, weight, num_global_tokens):
    from concourse.bass_utils import run_bass_kernel_spmd

    nc, table_name, idx_name, out_name = _build()

    weight = np.ascontiguousarray(np.asarray(weight), dtype=np.float32)
    ids = np.asarray(input_).reshape(-1).astype(np.int64)
    n = ids.shape[0]

    order = np.argsort(ids, kind="stable")
    sids = ids[order]

    # Greedy cut: up to C ids per chunk, id span < R.
    cuts = []
    s = 0
    while s < n:
        e = min(s + C, int(np.searchsorted(sids, sids[s] + R, side="left")))
        cuts.append((s, e))
        s = e

    out_flat = np.empty((n, D), np.float32)
    dummy_idx = _wrap_idxs(np.zeros(1, np.int16))

    for b in range(0, len(cuts), N_CORES):
        batch = cuts[b:b + N_CORES]
        in_maps = []
        for m in range(N_CORES):
            if m < len(batch):
                s, e = batch[m]
                base = min(int(sids[s]), VOCAB - R)
                in_maps.append({
                    table_name: weight[base:base + R],
                    idx_name: _wrap_idxs((sids[s:e] - base).astype(np.int16)),
                })
            else:
                in_maps.append({table_name: weight[:R], idx_name: dummy_idx})
        res = run_bass_kernel_spmd(nc, in_maps, core_ids=list(range(N_CORES)))
        for m, (s, e) in enumerate(batch):
            arr = res.results[m][out_name]          # [128, C/128, D]
            rows = arr.transpose(1, 0, 2).reshape(C, D)
            out_flat[order[s:e]] = rows[: e - s]

    return np.ascontiguousarray(out_flat.reshape(G, T, D).transpose(1, 0, 2))


# revision 3
# speedup vs baseline: 1.1819x; 1.1819x over previous
"""Distributed embedding gather (OEPEmbedding) on 8 Trainium2 NeuronCores.

Strategy (expert-parallel-style dispatch/combine, host-routed):
  - Deduplicate + sort all G*T token ids (np.unique); cut the sorted
    unique-id list into 8 equal chunks (perfect load balance). Each core
    receives a contiguous slice of the embedding table covering its
    chunk's id span (R rows, indexable by int16 local ids as the HW
    dma_gather instruction requires), plus the local ids in dma_gather's
    wrapped [128, C/16] layout.
  - On device, each core runs chunked dma_gather (HBM table -> SBUF)
    pipelined with HWDGE writes (SBUF -> HBM output). Each unique row is
    gathered exactly once.
  - Host expands duplicates / inverts the sort permutation to assemble
    the [T, G, D] output.

Degenerate id distributions that don't fit 8 span-limited chunks fall back
to additional SPMD launches (never happens for uniform ids).
"""

import numpy as np

VOCAB = 128000
D = 2048
G = 3
T = 16384
N = G * T            # 49152 total lookups
N_CORES = 8
C = 5376             # unique rows gathered per core (SPMD capacity)
R = 18048            # table rows staged per core (max id span per chunk)
CHUNK = 768          # ids per dma_gather instruction
P = 128
CPP = CHUNK // P     # rows per partition per chunk = 6
N_CHUNKS = C // CHUNK

_cached = None


def _build():
    """Build + compile the SPMD Bass program once per process."""
    global _cached
    if _cached is not None:
        return _cached
    import concourse.bacc as bacc
    import concourse.mybir as mybir
    import concourse.tile as tile

    nc = bacc.Bacc("TRN2", target_bir_lowering=False, debug=False)
    with tile.TileContext(nc) as tc:
        with tc.tile_pool(name="dram", bufs=1, space="DRAM") as dram:
            table = dram.tile([R, D], mybir.dt.float32, kind="ExternalInput")
            idxs = dram.tile([P, C // 16], mybir.dt.int16, kind="ExternalInput")
            out = dram.tile([P, N_CHUNKS * CPP, D], mybir.dt.float32,
                            kind="ExternalOutput")
            with (
                tc.tile_pool(name="idxp", bufs=N_CHUNKS) as idxp,
                tc.tile_pool(name="gbuf", bufs=3) as gbuf,
            ):
                cols = CHUNK // 16  # idx columns per chunk
                for k in range(N_CHUNKS):
                    idx_sb = idxp.tile([P, cols], mybir.dt.int16)
                    nc.sync.dma_start(idx_sb[:], idxs[:, k * cols:(k + 1) * cols])
                    t = gbuf.tile([P, CPP, D], mybir.dt.float32)
                    nc.gpsimd.dma_gather(
                        t[:], table[:], idx_sb[:, :], CHUNK, CHUNK, D,
                    )
                    nc.sync.dma_start(out[:, k * CPP:(k + 1) * CPP, :], t[:])
    nc.compile()
    _cached = (nc, table.name, idxs.name, out.name)
    return _cached


def _wrap_idxs(local_ids):
    """dma_gather idx layout: idx i at [i%16, i//16], replicated to 128 parts."""
    padded = np.zeros(C, np.int16)
    padded[: len(local_ids)] = local_ids
    w16 = padded.reshape(C // 16, 16).T  # [16, C/16]
    return np.ascontiguousarray(np.tile(w16, (8, 1)))  # [128, C/16]


def kernel(input_, weight, num_global_tokens):
    from concourse.bass_utils import run_bass_kernel_spmd

    nc, table_name, idx_name, out_name = _build()

    weight = np.ascontiguousarray(np.asarray(weight), dtype=np.float32)
    ids = np.asarray(input_).reshape(-1).astype(np.int64)

    uniq, inv = np.unique(ids, return_inverse=True)
    nu = len(uniq)

    # Greedy cut of the sorted unique-id list: id span < R per chunk, with
    # chunk sizes balanced across each batch of 8 cores (capacity C).
    cuts = []
    s = 0
    while s < nu:
        left_in_batch = N_CORES - (len(cuts) % N_CORES)
        target = min(C, -(-(nu - s) // left_in_batch))
        e = min(s + target, int(np.searchsorted(uniq, uniq[s] + R, side="left")))
        cuts.append((s, e))
        s = e

    uniq_rows = np.empty((nu, D), np.float32)
    dummy_idx = _wrap_idxs(np.zeros(1, np.int16))

    for b in range(0, len(cuts), N_CORES):
        batch = cuts[b:b + N_CORES]
        in_maps = []
        for m in range(N_CORES):
            if m < len(batch):
                s, e = batch[m]
                base = min(int(uniq[s]), VOCAB - R)
                in_maps.append({
                    table_name: weight[base:base + R],
                    idx_name: _wrap_idxs((uniq[s:e] - base).astype(np.int16)),
                })
            else:
                in_maps.append({table_name: weight[:R], idx_name: dummy_idx})
        res = run_bass_kernel_spmd(nc, in_maps, core_ids=list(range(N_CORES)))
        for m, (s, e) in enumerate(batch):
            arr = res.results[m][out_name]          # [128, C/128, D]
            rows = arr.transpose(1, 0, 2).reshape(C, D)
            uniq_rows[s:e] = rows[: e - s]

    out_flat = uniq_rows[inv]
    return np.ascontiguousarray(out_flat.reshape(G, T, D).transpose(1, 0, 2))


def make_in_maps(input_, weight):
    """Build the 8 per-core in_maps for the balanced single-launch case
    (helper for profiling in test.py)."""
    nc, table_name, idx_name, out_name = _build()
    weight = np.ascontiguousarray(np.asarray(weight), dtype=np.float32)
    ids = np.asarray(input_).reshape(-1).astype(np.int64)
    uniq, _ = np.unique(ids, return_inverse=True)
    nu = len(uniq)
    per = (nu + N_CORES - 1) // N_CORES
    in_maps = []
    for m in range(N_CORES):
        s, e = m * per, min((m + 1) * per, nu)
        base = min(int(uniq[s]), VOCAB - R)
        assert int(uniq[e - 1]) - base < R
        in_maps.append({
            table_name: weight[base:base + R],
            idx_name: _wrap_idxs((uniq[s:e] - base).astype(np.int16)),
        })
    return in_maps
